# revision 26
# baseline (speedup 1.0000x reference)
"""Additive attention (B=64, L=Q=K=H=1024) on 8 TRN2 NeuronCores.

Data-parallel over batch: each core owns 8 batches, no collectives.
Per batch the dominant op is kT[h,l] = sum_k W2[h,k]*keys[l,k], done on
TensorE in fp16 (1 cycle/row warm).  tanh(q+k) is fused into one ScalarE
pass (bias=q column); s = v . tanh(...) accumulates on TensorE, with each
s-matmul deferred one group so the in-order PE never waits on ScalarE.
The masked softmax runs on partition 0.  context = w @ keys reuses the
SAME [K,L]-layout keys tiles on the otherwise-idle VectorE: w is
partition-broadcast, then per 128-row tile a multiply + free-dim reduce
contracts L, writing context transposed; a strided DMA scatters it into
the output row.  Each batch's tail is emitted in the middle of the next
batch's j-loop so it hides under the main matmul stream.
"""

import sys

import numpy as np

_REPO = "/opt/trn_rl_repo"

B, L, Q, K, H = 64, 1024, 1024, 1024, 1024
NCORES = 8
BL = B // NCORES

_CACHE = {}


def _build(BL=BL, L=L, Q=Q, K=K, H=H, FC=512):
    if _REPO not in sys.path:
        sys.path.insert(0, _REPO)
    import concourse.tile as tile
    from concourse import bacc, mybir

    f32 = mybir.dt.float32
    f16 = mybir.dt.float16
    Tanh = mybir.ActivationFunctionType.Tanh
    Exp = mybir.ActivationFunctionType.Exp
    Copy = mybir.ActivationFunctionType.Copy

    PT = 128
    FC = min(FC, L, K)
    nkt, nht, nlt, nqt = K // PT, H // PT, L // PT, Q // PT
    nlc = L // FC

    nc = bacc.Bacc(None, target_bir_lowering=False)
    keysT = nc.declare_dram_parameter("keysT", [BL, K, L], f16, isOutput=False)
    w2t = nc.declare_dram_parameter("w2t", [K, H], f16, isOutput=False)
    w1t = nc.declare_dram_parameter("w1t", [Q, H], f16, isOutput=False)
    qryT = nc.declare_dram_parameter("qryT", [Q, BL], f16, isOutput=False)
    vT = nc.declare_dram_parameter("vT", [PT, H // 128], f16, isOutput=False)
    madd = nc.declare_dram_parameter("madd", [BL, L], f32, isOutput=False)
    keysNL = nc.declare_dram_parameter("keysNL", [L, K], f16, isOutput=False)
    out_ctx = nc.declare_dram_parameter("out_ctx", [BL, K], f32, isOutput=True)
    out_w = nc.declare_dram_parameter("out_w", [BL, L], f32, isOutput=True)

    with tile.TileContext(nc) as tc:
        with (
            tc.tile_pool(name="const", bufs=1) as constp,
            tc.tile_pool(name="keys", bufs=32) as keysp,
            tc.tile_pool(name="tt", bufs=5) as tp,
            tc.tile_pool(name="prod", bufs=2) as prodp,
            tc.tile_pool(name="small", bufs=2) as smallp,
            tc.tile_pool(name="psk", bufs=4, space="PSUM") as psk,
            tc.tile_pool(name="pss", bufs=4, space="PSUM") as pss,
        ):
            # ---- prologue DMAs, ordered so the q-projection and the first
            # main matmul group unblock as early as possible
            w2t_t = [
                constp.tile([PT, H], f16, tag=f"w2t{kt}", name=f"w2t_{kt}")
                for kt in range(nkt)
            ]
            kT_tiles = {}
            kT_tiles[0] = [
                keysp.tile([PT, L], f16, tag="kt", name=f"kT_0_{kt}")
                for kt in range(nkt)
            ]
            w1t_t = [
                keysp.tile([PT, H], f16, tag="kt", name=f"w1t_{qt}")
                for qt in range(nqt)
            ]
            vT_sb = constp.tile([PT, nht], f16)
            nc.sync.dma_start(vT_sb[:], vT[:])
            qryT_sb = constp.tile([PT, nqt, BL], f16)
            for qt in range(nqt):
                nc.sync.dma_start(qryT_sb[:, qt, :], qryT[qt * PT : (qt + 1) * PT, :])
            for qt in range(nqt):
                nc.sync.dma_start(w1t_t[qt][:], w1t[qt * PT : (qt + 1) * PT, :])
            for kt in range(nkt):
                nc.sync.dma_start(
                    kT_tiles[0][kt][:, 0:FC], keysT[0, kt * PT : (kt + 1) * PT, 0:FC]
                )
                nc.sync.dma_start(w2t_t[kt][:], w2t[kt * PT : (kt + 1) * PT, :])
            qT_sb = constp.tile([PT, nht, BL], f32)
            ident = constp.tile([1, 1], f32)
            nc.gpsimd.memset(ident[:], 1.0)


            state = {}
            extra = {}

            def emit_q():
                for j in range(nht):
                    qps = pss.tile([PT, BL], f32, tag="sps", name=f"qps_{j}")
                    for qt in range(nqt):
                        nc.tensor.matmul(
                            qps[:],
                            w1t_t[qt][:, j * PT : (j + 1) * PT],
                            qryT_sb[:, qt, :],
                            start=(qt == 0),
                            stop=(qt == nqt - 1),
                        )
                    nc.vector.tensor_copy(qT_sb[:, j, :], qps[:])

            def emit_tail(b):
                """softmax + VectorE context for batch b."""
                s_ps, madd_sb, kT_sb = state.pop(b)

                s_sb = smallp.tile([1, L], f32, tag="s", name=f"s_sb_{b}")
                for c in range(nlc):
                    nc.vector.tensor_add(
                        s_sb[:, c * FC : (c + 1) * FC],
                        s_ps[c][:, :FC],
                        madd_sb[:, c * FC : (c + 1) * FC],
                    )
                nmax = smallp.tile([1, 1], f32, tag="nmax", name=f"nmax_{b}")
                nc.vector.tensor_reduce(
                    nmax[:],
                    s_sb[:],
                    axis=mybir.AxisListType.X,
                    op=mybir.AluOpType.max,
                    negate=True,
                )
                e_sb = smallp.tile([1, L], f32, tag="e", name=f"e_sb_{b}")
                ssum = smallp.tile([1, 1], f32, tag="ssum", name=f"ssum_{b}")
                nc.scalar.activation(
                    e_sb[:], s_sb[:], Exp, bias=nmax[:], accum_out=ssum[:]
                )
                rinv = smallp.tile([1, 1], f32, tag="rinv", name=f"rinv_{b}")
                nc.vector.reciprocal(rinv[:], ssum[:])
                w_sb = smallp.tile([1, L], f32, tag="w", name=f"w_sb_{b}")
                nc.scalar.activation(w_sb[:], e_sb[:], Copy, scale=rinv[:])
                nc.sync.dma_start(out_w[b : b + 1, :], w_sb[:])

                w16 = smallp.tile([1, L], f16, tag="w16", name=f"w16_{b}")
                nc.scalar.activation(w16[:], e_sb[:], Copy, scale=rinv[:])
                if b < BL - 1:
                    # broadcast w across partitions; contract l on VectorE
                    wb = smallp.tile([PT, L], f16, tag="wb", name=f"wb_{b}")
                    nc.gpsimd.partition_broadcast(wb[:], w16[:])
                    ctxT = smallp.tile([PT, nkt], f32, tag="ctxT", name=f"ctxT_{b}")
                    for kt in range(nkt):
                        prod = prodp.tile(
                            [PT, L], f16, tag="prod", name=f"prod_{b}_{kt}"
                        )
                        nc.vector.tensor_mul(prod[:], kT_sb[kt][:], wb[:])
                        nc.vector.tensor_reduce(
                            ctxT[:, kt : kt + 1],
                            prod[:],
                            axis=mybir.AxisListType.X,
                            op=mybir.AluOpType.add,
                        )
                    nc.sync.dma_start(
                        out_ctx[b : b + 1, :].rearrange(
                            "a (kt p) -> (a p) kt", kt=nkt, p=PT
                        ),
                        ctxT[:],
                    )
                else:
                    # final batch: PE is idle by now.  Transpose the masked
                    # scores on the PE, exp into fp16 (bias = broadcast -max),
                    # matmul the unnormalized weights vs natural keys, and
                    # fold the 1/sum into the PSUM evacuation.
                    sT_ps = pss.tile([PT, nlt], f32, tag="sps", name="sT_ps_L")
                    for lt in range(nlt):
                        nc.tensor.transpose(
                            sT_ps[:, lt : lt + 1],
                            s_sb[0:1, lt * PT : (lt + 1) * PT],
                            ident[:],
                        )
                    nmaxb = smallp.tile([PT, 1], f32, tag="nmaxb", name="nmaxb_L")
                    nc.gpsimd.partition_broadcast(nmaxb[:], nmax[:])
                    eT = smallp.tile([PT, nlt], f16, tag="eT", name="eT_L")
                    nc.scalar.activation(eT[:], sT_ps[:], Exp, bias=nmaxb[:])
                    kN_sb = extra.pop("kN")
                    ctx_sb = smallp.tile([1, K], f32, tag="ctx", name="ctx_sb_L")
                    for c in range(K // FC):
                        cps = pss.tile([1, 512], f32, tag="sps", name=f"c_ps_{c}")
                        for lt in range(nlt):
                            nc.tensor.matmul(
                                cps[:, :FC],
                                eT[:, lt : lt + 1],
                                kN_sb[lt][:, c * FC : (c + 1) * FC],
                                start=(lt == 0),
                                stop=(lt == nlt - 1),
                            )
                        nc.vector.tensor_scalar_mul(
                            ctx_sb[:, c * FC : (c + 1) * FC], cps[:, :FC], rinv[:]
                        )
                    nc.sync.dma_start(out_ctx[b : b + 1, :], ctx_sb[:])

            emit_q()
            for b in range(BL):
                if b in kT_tiles:
                    kT_sb = kT_tiles[b]
                    # batch 0: chunk c=0 already requested above; fetch the rest
                    if FC < L:
                        for kt in range(nkt):
                            nc.sync.dma_start(
                                kT_sb[kt][:, FC:L],
                                keysT[b, kt * PT : (kt + 1) * PT, FC:L],
                            )
                else:
                    kT_sb = [
                        keysp.tile([PT, L], f16, tag="kt", name=f"kT_{b}_{kt}")
                        for kt in range(nkt)
                    ]
                    for kt in range(nkt):
                        nc.sync.dma_start(
                            kT_sb[kt][:], keysT[b, kt * PT : (kt + 1) * PT, :]
                        )
                madd_sb = smallp.tile([1, L], f32, tag="madd", name=f"madd_sb_{b}")
                nc.sync.dma_start(madd_sb[:], madd[b : b + 1, :])
                if b == BL - 1:
                    kN_sb = [
                        keysp.tile([PT, K], f16, tag="kt", name=f"kN_L_{lt}")
                        for lt in range(nlt)
                    ]
                    for lt in range(nlt):
                        nc.sync.dma_start(
                            kN_sb[lt][:], keysNL[lt * PT : (lt + 1) * PT, :]
                        )
                    extra["kN"] = kN_sb

                # s[l] = sum_h v[h] * tanh(q[h] + kT[h,l]); the s-matmul for a
                # group is emitted after the NEXT group's main matmuls so the
                # PE never waits on the ScalarE tanh.
                s_ps = [
                    pss.tile([1, 512], f32, tag="sps", name=f"s_ps_{b}_{c}")
                    for c in range(nlc)
                ]
                state[b] = (s_ps, madd_sb, kT_sb)
                pend = []
                for j in range(nht):
                    # both l-chunks accumulate in lockstep per kt so each j is
                    # one dense 16-matmul stream (fewer group boundaries, more
                    # slack for weight-load prefetch)
                    kps_c = [
                        psk.tile([PT, FC], f32, tag="kps", name=f"kps_{b}_{j}_{c}")
                        for c in range(nlc)
                    ]
                    for kt in range(nkt):
                        for c in range(nlc):
                            nc.tensor.matmul(
                                kps_c[c][:],
                                w2t_t[kt][:, j * PT : (j + 1) * PT],
                                kT_sb[kt][:, c * FC : (c + 1) * FC],
                                start=(kt == 0),
                                stop=(kt == nkt - 1),
                            )
                    while pend:
                        ptt, pj, pc = pend.pop(0)
                        nc.tensor.matmul(
                            s_ps[pc][:, :FC],
                            vT_sb[:, pj : pj + 1],
                            ptt[:],
                            start=(pj == 0),
                            stop=(pj == nht - 1),
                        )
                    for c in range(nlc):
                        tt = tp.tile([PT, FC], f16, tag="tt", name=f"tt_{b}_{j}_{c}")
                        nc.scalar.activation(
                            tt[:], kps_c[c][:], Tanh, bias=qT_sb[:, j, b : b + 1]
                        )
                        pend.append((tt, j, c))
                    if j == min(2, nht - 1) and (b - 1) in state:
                        for ptt, pj, pc in pend:
                            nc.tensor.matmul(
                                s_ps[pc][:, :FC],
                                vT_sb[:, pj : pj + 1],
                                ptt[:],
                                start=(pj == 0),
                                stop=(pj == nht - 1),
                            )
                        pend = []
                        emit_tail(b - 1)
                for ptt, pj, pc in pend:
                    nc.tensor.matmul(
                        s_ps[pc][:, :FC],
                        vT_sb[:, pj : pj + 1],
                        ptt[:],
                        start=(pj == 0),
                        stop=(pj == nht - 1),
                    )
            for rb in sorted(state):
                emit_tail(rb)

    nc.compile()
    return nc


def _shard_inputs(query, keys, mask, W1, W2, v):
    query = np.asarray(query, dtype=np.float32)
    keys = np.asarray(keys, dtype=np.float32)
    mask = np.asarray(mask)
    W1 = np.asarray(W1, dtype=np.float32)
    W2 = np.asarray(W2, dtype=np.float32)
    v = np.asarray(v, dtype=np.float32)

    w2t = np.ascontiguousarray(W2.T).astype(np.float16)
    w1t = np.ascontiguousarray(W1.T).astype(np.float16)
    vT = np.ascontiguousarray(v.reshape(H // 128, 128).T).astype(np.float16)
    madd = np.where(mask, np.float32(-1e30), np.float32(0.0))
    keys16 = keys.astype(np.float16)

    in_maps = []
    for i in range(NCORES):
        bs = slice(i * BL, (i + 1) * BL)
        in_maps.append(
            {
                "keysT": np.ascontiguousarray(keys16[bs].transpose(0, 2, 1)),
                "keysNL": np.ascontiguousarray(keys16[i * BL + BL - 1]),
                "w2t": w2t,
                "w1t": w1t,
                "qryT": np.ascontiguousarray(query[bs].T).astype(np.float16),
                "vT": vT,
                "madd": np.ascontiguousarray(madd[bs]),
            }
        )
    return in_maps


def kernel(query, keys, mask, W1, W2, v):
    if _REPO not in sys.path:
        sys.path.insert(0, _REPO)
    from concourse.bass_utils import run_bass_kernel_spmd

    if "nc" not in _CACHE:
        _CACHE["nc"] = _build()
    nc = _CACHE["nc"]

    in_maps = _shard_inputs(query, keys, mask, W1, W2, v)
    res = run_bass_kernel_spmd(nc, in_maps, core_ids=list(range(NCORES)))
    context = np.concatenate([res.results[i]["out_ctx"] for i in range(NCORES)], 0)
    weights = np.concatenate([res.results[i]["out_w"] for i in range(NCORES)], 0)
    return context, weights



# revision 27
# speedup vs baseline: 1.1690x; 1.1690x over previous
"""Additive attention (B=64, L=Q=K=H=1024) on 8 TRN2 NeuronCores.

Data-parallel over batch: each core owns 8 batches, no collectives.
Per batch the dominant op is kT[h,l] = sum_k W2[h,k]*keys[l,k], done on
TensorE in fp16 (1 cycle/row warm).  tanh(q+k) is fused into one ScalarE
pass (bias=q column); s = v . tanh(...) accumulates on TensorE, with each
s-matmul deferred one group so the in-order PE never waits on ScalarE.
The masked softmax runs on partition 0.  context = w @ keys reuses the
SAME [K,L]-layout keys tiles on the otherwise-idle VectorE: w is
partition-broadcast, then per 128-row tile a multiply + free-dim reduce
contracts L, writing context transposed; a strided DMA scatters it into
the output row.  Each batch's tail is emitted in the middle of the next
batch's j-loop so it hides under the main matmul stream.
"""

import sys

import numpy as np

_REPO = "/opt/trn_rl_repo"

B, L, Q, K, H = 64, 1024, 1024, 1024, 1024
NCORES = 8
BL = B // NCORES

_CACHE = {}


def _build(BL=BL, L=L, Q=Q, K=K, H=H, FC=512):
    if _REPO not in sys.path:
        sys.path.insert(0, _REPO)
    import concourse.tile as tile
    from concourse import bacc, mybir

    f32 = mybir.dt.float32
    f16 = mybir.dt.float16
    Tanh = mybir.ActivationFunctionType.Tanh
    Exp = mybir.ActivationFunctionType.Exp
    Copy = mybir.ActivationFunctionType.Copy

    PT = 128
    FC = min(FC, L, K)
    nkt, nht, nlt, nqt = K // PT, H // PT, L // PT, Q // PT
    nlc = L // FC

    nc = bacc.Bacc(None, target_bir_lowering=False)
    keysT = nc.declare_dram_parameter("keysT", [BL, K, L], f16, isOutput=False)
    w2t = nc.declare_dram_parameter("w2t", [K, H], f16, isOutput=False)
    w1t = nc.declare_dram_parameter("w1t", [Q, H], f16, isOutput=False)
    qryT = nc.declare_dram_parameter("qryT", [Q, BL], f16, isOutput=False)
    vT = nc.declare_dram_parameter("vT", [PT, H // 128], f16, isOutput=False)
    madd = nc.declare_dram_parameter("madd", [BL, L], f32, isOutput=False)
    keysNL = nc.declare_dram_parameter("keysNL", [L, K], f16, isOutput=False)
    out_ctx = nc.declare_dram_parameter("out_ctx", [BL, K], f32, isOutput=True)
    out_w = nc.declare_dram_parameter("out_w", [BL, L], f32, isOutput=True)

    with tile.TileContext(nc) as tc:
        with (
            tc.tile_pool(name="const", bufs=1) as constp,
            tc.tile_pool(name="keys", bufs=32) as keysp,
            tc.tile_pool(name="tt", bufs=5) as tp,
            tc.tile_pool(name="prod", bufs=2) as prodp,
            tc.tile_pool(name="small", bufs=2) as smallp,
            tc.tile_pool(name="psk", bufs=4, space="PSUM") as psk,
            tc.tile_pool(name="pss", bufs=4, space="PSUM") as pss,
        ):
            # ---- prologue DMAs, ordered so the q-projection and the first
            # main matmul group unblock as early as possible
            w2t_t = [
                constp.tile([PT, H], f16, tag=f"w2t{kt}", name=f"w2t_{kt}")
                for kt in range(nkt)
            ]
            kT_tiles = {}
            kT_tiles[0] = [
                keysp.tile([PT, L], f16, tag="kt", name=f"kT_0_{kt}")
                for kt in range(nkt)
            ]
            w1t_t = [
                keysp.tile([PT, H], f16, tag="kt", name=f"w1t_{qt}")
                for qt in range(nqt)
            ]
            vT_sb = constp.tile([PT, nht], f16)
            nc.sync.dma_start(vT_sb[:], vT[:])
            qryT_sb = constp.tile([PT, nqt, BL], f16)
            for qt in range(nqt):
                nc.sync.dma_start(qryT_sb[:, qt, :], qryT[qt * PT : (qt + 1) * PT, :])
            for qt in range(nqt):
                nc.sync.dma_start(w1t_t[qt][:], w1t[qt * PT : (qt + 1) * PT, :])
            for kt in range(nkt):
                nc.sync.dma_start(
                    kT_tiles[0][kt][:, 0:FC], keysT[0, kt * PT : (kt + 1) * PT, 0:FC]
                )
                nc.sync.dma_start(w2t_t[kt][:], w2t[kt * PT : (kt + 1) * PT, :])
            qT_sb = constp.tile([PT, nht, BL], f32)
            ident = constp.tile([1, 1], f32)
            nc.gpsimd.memset(ident[:], 1.0)


            state = {}
            extra = {}

            def emit_q():
                for j in range(nht):
                    qps = pss.tile([PT, BL], f32, tag="sps", name=f"qps_{j}")
                    for qt in range(nqt):
                        nc.tensor.matmul(
                            qps[:],
                            w1t_t[qt][:, j * PT : (j + 1) * PT],
                            qryT_sb[:, qt, :],
                            start=(qt == 0),
                            stop=(qt == nqt - 1),
                        )
                    nc.vector.tensor_copy(qT_sb[:, j, :], qps[:])

            def emit_tail(b):
                """softmax + VectorE context for batch b."""
                s_ps, madd_sb, kT_sb = state.pop(b)

                s_sb = smallp.tile([1, L], f32, tag="s", name=f"s_sb_{b}")
                for c in range(nlc):
                    nc.vector.tensor_add(
                        s_sb[:, c * FC : (c + 1) * FC],
                        s_ps[c][:, :FC],
                        madd_sb[:, c * FC : (c + 1) * FC],
                    )
                nmax = smallp.tile([1, 1], f32, tag="nmax", name=f"nmax_{b}")
                nc.vector.tensor_reduce(
                    nmax[:],
                    s_sb[:],
                    axis=mybir.AxisListType.X,
                    op=mybir.AluOpType.max,
                    negate=True,
                )
                e_sb = smallp.tile([1, L], f32, tag="e", name=f"e_sb_{b}")
                ssum = smallp.tile([1, 1], f32, tag="ssum", name=f"ssum_{b}")
                nc.scalar.activation(
                    e_sb[:], s_sb[:], Exp, bias=nmax[:], accum_out=ssum[:]
                )
                rinv = smallp.tile([1, 1], f32, tag="rinv", name=f"rinv_{b}")
                nc.vector.reciprocal(rinv[:], ssum[:])
                w_sb = smallp.tile([1, L], f32, tag="w", name=f"w_sb_{b}")
                nc.scalar.activation(w_sb[:], e_sb[:], Copy, scale=rinv[:])
                nc.sync.dma_start(out_w[b : b + 1, :], w_sb[:])

                w16 = smallp.tile([1, L], f16, tag="w16", name=f"w16_{b}")
                nc.scalar.activation(w16[:], e_sb[:], Copy, scale=rinv[:])
                if b < BL - 1:
                    # broadcast w across partitions; contract l on VectorE
                    wb = smallp.tile([PT, L], f16, tag="wb", name=f"wb_{b}")
                    nc.gpsimd.partition_broadcast(wb[:], w16[:])
                    ctxT = smallp.tile([PT, nkt], f32, tag="ctxT", name=f"ctxT_{b}")
                    for kt in range(nkt):
                        prod = prodp.tile(
                            [PT, L], f16, tag="prod", name=f"prod_{b}_{kt}"
                        )
                        nc.vector.tensor_mul(prod[:], kT_sb[kt][:], wb[:])
                        nc.vector.tensor_reduce(
                            ctxT[:, kt : kt + 1],
                            prod[:],
                            axis=mybir.AxisListType.X,
                            op=mybir.AluOpType.add,
                        )
                    nc.sync.dma_start(
                        out_ctx[b : b + 1, :].rearrange(
                            "a (kt p) -> (a p) kt", kt=nkt, p=PT
                        ),
                        ctxT[:],
                    )
                else:
                    # final batch: PE is idle by now.  Transpose the masked
                    # scores on the PE, exp into fp16 (bias = broadcast -max),
                    # matmul the unnormalized weights vs natural keys, and
                    # fold the 1/sum into the PSUM evacuation.
                    sT_ps = pss.tile([PT, nlt], f32, tag="sps", name="sT_ps_L")
                    for lt in range(nlt):
                        nc.tensor.transpose(
                            sT_ps[:, lt : lt + 1],
                            s_sb[0:1, lt * PT : (lt + 1) * PT],
                            ident[:],
                        )
                    nmaxb = smallp.tile([PT, 1], f32, tag="nmaxb", name="nmaxb_L")
                    nc.gpsimd.partition_broadcast(nmaxb[:], nmax[:])
                    eT = smallp.tile([PT, nlt], f16, tag="eT", name="eT_L")
                    nc.scalar.activation(eT[:], sT_ps[:], Exp, bias=nmaxb[:])
                    kN_sb = extra.pop("kN")
                    ctx_sb = smallp.tile([1, K], f32, tag="ctx", name="ctx_sb_L")
                    for c in range(K // FC):
                        cps = pss.tile([1, 512], f32, tag="sps", name=f"c_ps_{c}")
                        for lt in range(nlt):
                            nc.tensor.matmul(
                                cps[:, :FC],
                                eT[:, lt : lt + 1],
                                kN_sb[lt][:, c * FC : (c + 1) * FC],
                                start=(lt == 0),
                                stop=(lt == nlt - 1),
                            )
                        nc.vector.tensor_scalar_mul(
                            ctx_sb[:, c * FC : (c + 1) * FC], cps[:, :FC], rinv[:]
                        )
                    nc.sync.dma_start(out_ctx[b : b + 1, :], ctx_sb[:])

            emit_q()
            for b in range(BL):
                if b in kT_tiles:
                    kT_sb = kT_tiles[b]
                    # batch 0: chunk c=0 already requested above; fetch the rest
                    if FC < L:
                        for kt in range(nkt):
                            nc.sync.dma_start(
                                kT_sb[kt][:, FC:L],
                                keysT[b, kt * PT : (kt + 1) * PT, FC:L],
                            )
                else:
                    kT_sb = [
                        keysp.tile([PT, L], f16, tag="kt", name=f"kT_{b}_{kt}")
                        for kt in range(nkt)
                    ]
                    for kt in range(nkt):
                        nc.sync.dma_start(
                            kT_sb[kt][:], keysT[b, kt * PT : (kt + 1) * PT, :]
                        )
                madd_sb = smallp.tile([1, L], f32, tag="madd", name=f"madd_sb_{b}")
                nc.sync.dma_start(madd_sb[:], madd[b : b + 1, :])
                if b == BL - 1:
                    kN_sb = [
                        keysp.tile([PT, K], f16, tag="kt", name=f"kN_L_{lt}")
                        for lt in range(nlt)
                    ]
                    for lt in range(nlt):
                        nc.sync.dma_start(
                            kN_sb[lt][:], keysNL[lt * PT : (lt + 1) * PT, :]
                        )
                    extra["kN"] = kN_sb

                # s[l] = sum_h v[h] * tanh(q[h] + kT[h,l]); the s-matmul for a
                # group is emitted after the NEXT group's main matmuls so the
                # PE never waits on the ScalarE tanh.
                s_ps = [
                    pss.tile([1, 512], f32, tag="sps", name=f"s_ps_{b}_{c}")
                    for c in range(nlc)
                ]
                state[b] = (s_ps, madd_sb, kT_sb)
                pend = []
                for j in range(nht):
                    for c in range(nlc):
                        kps = psk.tile(
                            [PT, FC], f32, tag="kps", name=f"kps_{b}_{j}_{c}"
                        )
                        for kt in range(nkt):
                            nc.tensor.matmul(
                                kps[:],
                                w2t_t[kt][:, j * PT : (j + 1) * PT],
                                kT_sb[kt][:, c * FC : (c + 1) * FC],
                                start=(kt == 0),
                                stop=(kt == nkt - 1),
                            )

                        if len(pend) >= 2:
                            ptt, pj, pc = pend.pop(0)
                            nc.tensor.matmul(
                                s_ps[pc][:, :FC],
                                vT_sb[:, pj : pj + 1],
                                ptt[:],
                                start=(pj == 0),
                                stop=(pj == nht - 1),
                            )
                        tt = tp.tile([PT, FC], f16, tag="tt", name=f"tt_{b}_{j}_{c}")
                        nc.scalar.activation(
                            tt[:], kps[:], Tanh, bias=qT_sb[:, j, b : b + 1]
                        )
                        pend.append((tt, j, c))
                    if j == min(2, nht - 1) and (b - 1) in state:
                        for ptt, pj, pc in pend:
                            nc.tensor.matmul(
                                s_ps[pc][:, :FC],
                                vT_sb[:, pj : pj + 1],
                                ptt[:],
                                start=(pj == 0),
                                stop=(pj == nht - 1),
                            )
                        pend = []
                        emit_tail(b - 1)
                for ptt, pj, pc in pend:
                    nc.tensor.matmul(
                        s_ps[pc][:, :FC],
                        vT_sb[:, pj : pj + 1],
                        ptt[:],
                        start=(pj == 0),
                        stop=(pj == nht - 1),
                    )
            for rb in sorted(state):
                emit_tail(rb)

    nc.compile()
    return nc


def _shard_inputs(query, keys, mask, W1, W2, v):
    query = np.asarray(query, dtype=np.float32)
    keys = np.asarray(keys, dtype=np.float32)
    mask = np.asarray(mask)
    W1 = np.asarray(W1, dtype=np.float32)
    W2 = np.asarray(W2, dtype=np.float32)
    v = np.asarray(v, dtype=np.float32)

    w2t = np.ascontiguousarray(W2.T).astype(np.float16)
    w1t = np.ascontiguousarray(W1.T).astype(np.float16)
    vT = np.ascontiguousarray(v.reshape(H // 128, 128).T).astype(np.float16)
    madd = np.where(mask, np.float32(-1e30), np.float32(0.0))
    keys16 = keys.astype(np.float16)

    in_maps = []
    for i in range(NCORES):
        bs = slice(i * BL, (i + 1) * BL)
        in_maps.append(
            {
                "keysT": np.ascontiguousarray(keys16[bs].transpose(0, 2, 1)),
                "keysNL": np.ascontiguousarray(keys16[i * BL + BL - 1]),
                "w2t": w2t,
                "w1t": w1t,
                "qryT": np.ascontiguousarray(query[bs].T).astype(np.float16),
                "vT": vT,
                "madd": np.ascontiguousarray(madd[bs]),
            }
        )
    return in_maps


def kernel(query, keys, mask, W1, W2, v):
    if _REPO not in sys.path:
        sys.path.insert(0, _REPO)
    from concourse.bass_utils import run_bass_kernel_spmd

    if "nc" not in _CACHE:
        _CACHE["nc"] = _build()
    nc = _CACHE["nc"]

    in_maps = _shard_inputs(query, keys, mask, W1, W2, v)
    res = run_bass_kernel_spmd(nc, in_maps, core_ids=list(range(NCORES)))
    context = np.concatenate([res.results[i]["out_ctx"] for i in range(NCORES)], 0)
    weights = np.concatenate([res.results[i]["out_w"] for i in range(NCORES)], 0)
    return context, weights



# revision 28
# speedup vs baseline: 1.2533x; 1.0722x over previous
"""Additive attention (B=64, L=Q=K=H=1024) on 8 TRN2 NeuronCores.

Data-parallel over batch: each core owns 8 batches, no collectives.
Per batch the dominant op is kT[h,l] = sum_k W2[h,k]*keys[l,k], done on
TensorE in fp16 (1 cycle/row warm).  tanh(q+k) is fused into one ScalarE
pass (bias=q column); s = v . tanh(...) accumulates on TensorE, with each
s-matmul deferred one group so the in-order PE never waits on ScalarE.
The masked softmax runs on partition 0.  context = w @ keys reuses the
SAME [K,L]-layout keys tiles on the otherwise-idle VectorE: w is
partition-broadcast, then per 128-row tile a multiply + free-dim reduce
contracts L, writing context transposed; a strided DMA scatters it into
the output row.  Each batch's tail is emitted in the middle of the next
batch's j-loop so it hides under the main matmul stream.
"""

import sys

import numpy as np

_REPO = "/opt/trn_rl_repo"

B, L, Q, K, H = 64, 1024, 1024, 1024, 1024
NCORES = 8
BL = B // NCORES

_CACHE = {}


def _build(BL=BL, L=L, Q=Q, K=K, H=H, FC=512):
    if _REPO not in sys.path:
        sys.path.insert(0, _REPO)
    import concourse.tile as tile
    from concourse import bacc, mybir

    f32 = mybir.dt.float32
    f16 = mybir.dt.float16
    Tanh = mybir.ActivationFunctionType.Tanh
    Exp = mybir.ActivationFunctionType.Exp
    Copy = mybir.ActivationFunctionType.Copy

    PT = 128
    FC = min(FC, L, K)
    nkt, nht, nlt, nqt = K // PT, H // PT, L // PT, Q // PT
    nlc = L // FC

    nc = bacc.Bacc(None, target_bir_lowering=False)
    keysT = nc.declare_dram_parameter("keysT", [BL, K, L], f16, isOutput=False)
    w2t = nc.declare_dram_parameter("w2t", [K, H], f16, isOutput=False)
    w1t = nc.declare_dram_parameter("w1t", [Q, H], f16, isOutput=False)
    qryT = nc.declare_dram_parameter("qryT", [Q, BL], f16, isOutput=False)
    vT = nc.declare_dram_parameter("vT", [PT, H // 128], f16, isOutput=False)
    madd = nc.declare_dram_parameter("madd", [BL, L], f32, isOutput=False)
    keysNL = nc.declare_dram_parameter("keysNL", [L, K], f16, isOutput=False)
    out_ctx = nc.declare_dram_parameter("out_ctx", [BL, K], f32, isOutput=True)
    out_w = nc.declare_dram_parameter("out_w", [BL, L], f32, isOutput=True)

    with tile.TileContext(nc) as tc:
        with (
            tc.tile_pool(name="const", bufs=1) as constp,
            tc.tile_pool(name="keys", bufs=32) as keysp,
            tc.tile_pool(name="tt", bufs=20) as tp,
            tc.tile_pool(name="prod", bufs=2) as prodp,
            tc.tile_pool(name="small", bufs=2) as smallp,
            tc.tile_pool(name="psk", bufs=4, space="PSUM") as psk,
            tc.tile_pool(name="pss", bufs=4, space="PSUM") as pss,
        ):
            # ---- prologue DMAs, ordered so the q-projection and the first
            # main matmul group unblock as early as possible
            w2t_t = [
                constp.tile([PT, H], f16, tag=f"w2t{kt}", name=f"w2t_{kt}")
                for kt in range(nkt)
            ]
            kT_tiles = {}
            kT_tiles[0] = [
                keysp.tile([PT, L], f16, tag="kt", name=f"kT_0_{kt}")
                for kt in range(nkt)
            ]
            w1t_t = [
                keysp.tile([PT, H], f16, tag="kt", name=f"w1t_{qt}")
                for qt in range(nqt)
            ]
            vT_sb = constp.tile([PT, nht], f16)
            nc.sync.dma_start(vT_sb[:], vT[:])
            qryT_sb = constp.tile([PT, nqt, BL], f16)
            for qt in range(nqt):
                nc.sync.dma_start(qryT_sb[:, qt, :], qryT[qt * PT : (qt + 1) * PT, :])
            for qt in range(nqt):
                nc.sync.dma_start(w1t_t[qt][:], w1t[qt * PT : (qt + 1) * PT, :])
            for kt in range(nkt):
                nc.sync.dma_start(
                    kT_tiles[0][kt][:, 0:FC], keysT[0, kt * PT : (kt + 1) * PT, 0:FC]
                )
                nc.sync.dma_start(w2t_t[kt][:], w2t[kt * PT : (kt + 1) * PT, :])
            qT_sb = constp.tile([PT, nht, BL], f32)
            ident = constp.tile([1, 1], f32)
            nc.gpsimd.memset(ident[:], 1.0)


            state = {}
            extra = {}

            def emit_q():
                for j in range(nht):
                    qps = pss.tile([PT, BL], f32, tag="sps", name=f"qps_{j}")
                    for qt in range(nqt):
                        nc.tensor.matmul(
                            qps[:],
                            w1t_t[qt][:, j * PT : (j + 1) * PT],
                            qryT_sb[:, qt, :],
                            start=(qt == 0),
                            stop=(qt == nqt - 1),
                        )
                    nc.vector.tensor_copy(qT_sb[:, j, :], qps[:])

            def emit_tail(b):
                """softmax + VectorE context for batch b."""
                s_ps, madd_sb, kT_sb = state.pop(b)

                s_sb = smallp.tile([1, L], f32, tag="s", name=f"s_sb_{b}")
                for c in range(nlc):
                    nc.vector.tensor_add(
                        s_sb[:, c * FC : (c + 1) * FC],
                        s_ps[c][:, :FC],
                        madd_sb[:, c * FC : (c + 1) * FC],
                    )
                nmax = smallp.tile([1, 1], f32, tag="nmax", name=f"nmax_{b}")
                nc.vector.tensor_reduce(
                    nmax[:],
                    s_sb[:],
                    axis=mybir.AxisListType.X,
                    op=mybir.AluOpType.max,
                    negate=True,
                )
                e_sb = smallp.tile([1, L], f32, tag="e", name=f"e_sb_{b}")
                ssum = smallp.tile([1, 1], f32, tag="ssum", name=f"ssum_{b}")
                nc.scalar.activation(
                    e_sb[:], s_sb[:], Exp, bias=nmax[:], accum_out=ssum[:]
                )
                rinv = smallp.tile([1, 1], f32, tag="rinv", name=f"rinv_{b}")
                nc.vector.reciprocal(rinv[:], ssum[:])
                w_sb = smallp.tile([1, L], f32, tag="w", name=f"w_sb_{b}")
                nc.scalar.activation(w_sb[:], e_sb[:], Copy, scale=rinv[:])
                nc.sync.dma_start(out_w[b : b + 1, :], w_sb[:])

                w16 = smallp.tile([1, L], f16, tag="w16", name=f"w16_{b}")
                nc.scalar.activation(w16[:], e_sb[:], Copy, scale=rinv[:])
                if b < BL - 1:
                    # broadcast w across partitions; contract l on VectorE
                    wb = smallp.tile([PT, L], f16, tag="wb", name=f"wb_{b}")
                    nc.gpsimd.partition_broadcast(wb[:], w16[:])
                    ctxT = smallp.tile([PT, nkt], f32, tag="ctxT", name=f"ctxT_{b}")
                    for kt in range(nkt):
                        prod = prodp.tile(
                            [PT, L], f16, tag="prod", name=f"prod_{b}_{kt}"
                        )
                        nc.vector.tensor_mul(prod[:], kT_sb[kt][:], wb[:])
                        nc.vector.tensor_reduce(
                            ctxT[:, kt : kt + 1],
                            prod[:],
                            axis=mybir.AxisListType.X,
                            op=mybir.AluOpType.add,
                        )
                    nc.sync.dma_start(
                        out_ctx[b : b + 1, :].rearrange(
                            "a (kt p) -> (a p) kt", kt=nkt, p=PT
                        ),
                        ctxT[:],
                    )
                else:
                    # final batch: PE is idle by now.  Transpose the masked
                    # scores on the PE, exp into fp16 (bias = broadcast -max),
                    # matmul the unnormalized weights vs natural keys, and
                    # fold the 1/sum into the PSUM evacuation.
                    sT_ps = pss.tile([PT, nlt], f32, tag="sps", name="sT_ps_L")
                    for lt in range(nlt):
                        nc.tensor.transpose(
                            sT_ps[:, lt : lt + 1],
                            s_sb[0:1, lt * PT : (lt + 1) * PT],
                            ident[:],
                        )
                    nmaxb = smallp.tile([PT, 1], f32, tag="nmaxb", name="nmaxb_L")
                    nc.gpsimd.partition_broadcast(nmaxb[:], nmax[:])
                    eT = smallp.tile([PT, nlt], f16, tag="eT", name="eT_L")
                    nc.scalar.activation(eT[:], sT_ps[:], Exp, bias=nmaxb[:])
                    kN_sb = extra.pop("kN")
                    ctx_sb = smallp.tile([1, K], f32, tag="ctx", name="ctx_sb_L")
                    for c in range(K // FC):
                        cps = pss.tile([1, 512], f32, tag="sps", name=f"c_ps_{c}")
                        for lt in range(nlt):
                            nc.tensor.matmul(
                                cps[:, :FC],
                                eT[:, lt : lt + 1],
                                kN_sb[lt][:, c * FC : (c + 1) * FC],
                                start=(lt == 0),
                                stop=(lt == nlt - 1),
                            )
                        nc.vector.tensor_scalar_mul(
                            ctx_sb[:, c * FC : (c + 1) * FC], cps[:, :FC], rinv[:]
                        )
                    nc.sync.dma_start(out_ctx[b : b + 1, :], ctx_sb[:])

            emit_q()
            for b in range(BL):
                if b in kT_tiles:
                    kT_sb = kT_tiles[b]
                    # batch 0: chunk c=0 already requested above; fetch the rest
                    if FC < L:
                        for kt in range(nkt):
                            nc.sync.dma_start(
                                kT_sb[kt][:, FC:L],
                                keysT[b, kt * PT : (kt + 1) * PT, FC:L],
                            )
                else:
                    kT_sb = [
                        keysp.tile([PT, L], f16, tag="kt", name=f"kT_{b}_{kt}")
                        for kt in range(nkt)
                    ]
                    for kt in range(nkt):
                        nc.sync.dma_start(
                            kT_sb[kt][:], keysT[b, kt * PT : (kt + 1) * PT, :]
                        )
                madd_sb = smallp.tile([1, L], f32, tag="madd", name=f"madd_sb_{b}")
                nc.sync.dma_start(madd_sb[:], madd[b : b + 1, :])
                if b == BL - 1:
                    kN_sb = [
                        keysp.tile([PT, K], f16, tag="kt", name=f"kN_L_{lt}")
                        for lt in range(nlt)
                    ]
                    for lt in range(nlt):
                        nc.sync.dma_start(
                            kN_sb[lt][:], keysNL[lt * PT : (lt + 1) * PT, :]
                        )
                    extra["kN"] = kN_sb

                # s[l] = sum_h v[h] * tanh(q[h] + kT[h,l]); the s-matmul for a
                # group is emitted after the NEXT group's main matmuls so the
                # PE never waits on the ScalarE tanh.
                s_ps = [
                    pss.tile([1, 512], f32, tag="sps", name=f"s_ps_{b}_{c}")
                    for c in range(nlc)
                ]
                state[b] = (s_ps, madd_sb, kT_sb)
                tts = {}
                for j in range(nht):
                    for c in range(nlc):
                        kps = psk.tile(
                            [PT, FC], f32, tag="kps", name=f"kps_{b}_{j}_{c}"
                        )
                        for kt in range(nkt):
                            nc.tensor.matmul(
                                kps[:],
                                w2t_t[kt][:, j * PT : (j + 1) * PT],
                                kT_sb[kt][:, c * FC : (c + 1) * FC],
                                start=(kt == 0),
                                stop=(kt == nkt - 1),
                            )
                        tt = tp.tile([PT, FC], f16, tag="tt", name=f"tt_{b}_{j}_{c}")
                        nc.scalar.activation(
                            tt[:], kps[:], Tanh, bias=qT_sb[:, j, b : b + 1]
                        )
                        tts[(j, c)] = tt
                    if j == min(2, nht - 1) and (b - 1) in state:
                        emit_tail(b - 1)
                # all s-matmuls as two clean single-bank runs at batch end:
                # keeps the main stream free of extra PSUM bank switches
                for c in range(nlc):
                    for j in range(nht):
                        nc.tensor.matmul(
                            s_ps[c][:, :FC],
                            vT_sb[:, j : j + 1],
                            tts[(j, c)][:],
                            start=(j == 0),
                            stop=(j == nht - 1),
                        )
            for rb in sorted(state):
                emit_tail(rb)

    nc.compile()
    return nc


def _shard_inputs(query, keys, mask, W1, W2, v):
    query = np.asarray(query, dtype=np.float32)
    keys = np.asarray(keys, dtype=np.float32)
    mask = np.asarray(mask)
    W1 = np.asarray(W1, dtype=np.float32)
    W2 = np.asarray(W2, dtype=np.float32)
    v = np.asarray(v, dtype=np.float32)

    w2t = np.ascontiguousarray(W2.T).astype(np.float16)
    w1t = np.ascontiguousarray(W1.T).astype(np.float16)
    vT = np.ascontiguousarray(v.reshape(H // 128, 128).T).astype(np.float16)
    madd = np.where(mask, np.float32(-1e30), np.float32(0.0))
    keys16 = keys.astype(np.float16)

    in_maps = []
    for i in range(NCORES):
        bs = slice(i * BL, (i + 1) * BL)
        in_maps.append(
            {
                "keysT": np.ascontiguousarray(keys16[bs].transpose(0, 2, 1)),
                "keysNL": np.ascontiguousarray(keys16[i * BL + BL - 1]),
                "w2t": w2t,
                "w1t": w1t,
                "qryT": np.ascontiguousarray(query[bs].T).astype(np.float16),
                "vT": vT,
                "madd": np.ascontiguousarray(madd[bs]),
            }
        )
    return in_maps


def kernel(query, keys, mask, W1, W2, v):
    if _REPO not in sys.path:
        sys.path.insert(0, _REPO)
    from concourse.bass_utils import run_bass_kernel_spmd

    if "nc" not in _CACHE:
        _CACHE["nc"] = _build()
    nc = _CACHE["nc"]

    in_maps = _shard_inputs(query, keys, mask, W1, W2, v)
    res = run_bass_kernel_spmd(nc, in_maps, core_ids=list(range(NCORES)))
    context = np.concatenate([res.results[i]["out_ctx"] for i in range(NCORES)], 0)
    weights = np.concatenate([res.results[i]["out_w"] for i in range(NCORES)], 0)
    return context, weights

